# revision 28
# baseline (speedup 1.0000x reference)
"""Trainium2 Bass kernel for DequantingLinear (GGML Q8_0 block-dequant + linear).

y = x @ (w_q * scales).reshape(O, I).T + bias

Sharding: tensor-parallel over out_features across 8 NeuronCores; x replicated.
The Q8_0 dequant (w = scale * int8_quant) is a per-element multiply folded into
host-side prep (bf16 round-to-nearest — numerically identical to doing the same
bf16 multiply on the DVE, but off the device's critical path). Each core runs a
bf16 GEMM over its output-column shard, accumulating in fp32 PSUM.

Startup: slab 0's x arrives as 12 k-strips (1KB DMA packets — the DMA engine is
packet-rate-bound at ~0.5 pkt/ns, so 256B packets would halve bandwidth), and
phase 0 runs k-outer across all 8 PSUM banks so the PE consumes weight k-tiles
at the DMA delivery rate. A handful of warm-up matmuls on memset tiles start
the HAM activity window early so real matmuls issue at the 2.4 GHz warm clock.
Slabs 1-7 run the steady oc-inner sweep at the PE issue roofline.

Host-side prep:
  - x   [T, I] f32   -> xT    [I, T] bf16  (replicated; contraction on partitions)
  - w_q*scales       -> wT    [I, O/8] bf16 per core (dequantized W^T shard)
  - bias [O] f32     -> biasb [128, O/8] bf16 per core (partition-broadcast)
"""

import numpy as np
import ml_dtypes

# Problem shape (hardcoded per contest rules).
T = 4096          # tokens (matmul M)
I = 3072          # in_features (contraction K)
O = 12288         # out_features (matmul N)
BLOCK = 32
N_CORES = 8
OS = O // N_CORES  # 1536 out features per core
P = 128           # partitions
KT = I // P       # 24 k-tiles
NQ = 512          # psum free-dim quantum (one bank)
OCH = OS // NQ    # 3 o-chunks per core
TSLAB = 512       # t columns per steady x slab
NSLAB = T // TSLAB   # 8 slabs
TPS = TSLAB // P     # 4 t-tiles per slab
# k=0's weight tile and x strip ship as one host-concatenated "head" tensor
# (single DMA issue + transfer on the critical first-matmul path). Remaining
# slab-0 x arrives as k-strips: one single-tile strip, then 2-tile strips.
HEADW = OS + TSLAB
STRIP_KS = [2] * 11
STRIP_K0 = [2 + sum(STRIP_KS[:j]) for j in range(len(STRIP_KS))]

_CACHE = {}


def _strip_redundant_ldw(nc):
    """Tile lowering prepends an InstLdweights to every InstMatmult. Walk each
    block in scheduled order tracking the weights AP currently loaded in the
    PE array; an InstLdweights identical to the resident one is redundant --
    remove it, migrating its sync waits/updates onto the next instruction."""
    removed = 0
    for f in nc.m.functions:
        for bb in f.blocks:
            insts = bb.instructions
            drop = []
            last_w = None
            for idx, ins in enumerate(insts):
                tn = type(ins).__name__
                if tn == "InstLdweights":
                    key = repr(ins.ins[0])
                    nxt = insts[idx + 1] if idx + 1 < len(insts) else None
                    if (
                        key == last_w
                        and nxt is not None
                        and type(nxt).__name__ == "InstMatmult"
                    ):
                        si = ins.sync_info
                        if si is not None and (si.on_wait or si.on_update):
                            nsi = nxt.sync_info
                            if nsi is None:
                                nxt.sync_info = si
                            else:
                                nsi.on_wait = list(si.on_wait) + list(nsi.on_wait)
                                nsi.on_update = (
                                    list(nsi.on_update) + list(si.on_update)
                                )
                        drop.append(idx)
                    else:
                        last_w = key
            for idx in reversed(drop):
                del insts[idx]
            removed += len(drop)
    return removed


def _build():
    import concourse.bacc as bacc
    import concourse.mybir as mybir
    from concourse.tile import TileContext

    nc = bacc.Bacc("TRN2", num_devices=N_CORES)
    dt = mybir.dt

    xT = nc.declare_dram_parameter("xT", [I, T], dt.bfloat16, isOutput=False)
    wT = nc.declare_dram_parameter("wT", [I, OS], dt.bfloat16, isOutput=False)
    headp = nc.declare_dram_parameter(
        "head", [P, HEADW], dt.bfloat16, isOutput=False
    )
    head2p = nc.declare_dram_parameter(
        "head2", [P, HEADW], dt.bfloat16, isOutput=False
    )
    biasb = nc.declare_dram_parameter("biasb", [P, OS], dt.bfloat16, isOutput=False)
    y = nc.declare_dram_parameter("y", [T, OS], dt.float32, isOutput=True)

    PS_TAGS = ["ps0", "ps1", "ps2"]
    PS_BUFS = {"ps0": 3, "ps1": 3, "ps2": 2}   # 8 PSUM banks total

    with TileContext(nc) as tc:
        with (
            tc.tile_pool(name="wres", bufs=1) as wres,
            tc.tile_pool(name="xsl", bufs=2) as xsl,
            tc.tile_pool(name="outp", bufs=6) as outp,
            tc.tile_pool(name="psum", bufs=8, space="PSUM") as psum,
        ):
            xview = xT.rearrange("(k p) t -> p k t", p=P)

            def ps_tile(oc, name):
                tg = PS_TAGS[oc]
                return psum.tile(
                    [P, NQ], dt.float32, tag=tg, bufs=PS_BUFS[tg], name=name
                )

            def drain(s, tt, ps, oc):
                row = s * TSLAB + tt * P
                ot = outp.tile([P, NQ], dt.float32, tag="ot", name="ot")
                nc.vector.tensor_add(
                    ot[:, :], ps[:, :], biast[:, oc * NQ:(oc + 1) * NQ]
                )
                nc.sync.dma_start(
                    out=y[row:row + P, oc * NQ:(oc + 1) * NQ], in_=ot[:, :]
                )

            # ---- warm-up: start the PE HAM activity window early ----
            wmx = xsl.tile([P, P], dt.bfloat16, tag="wmx", bufs=1, name="wmx")
            nc.vector.memset(wmx[:, :], 0.0)
            wmw = xsl.tile([P, NQ], dt.bfloat16, tag="wmw", bufs=1, name="wmw")
            nc.vector.memset(wmw[:, :], 0.0)
            wmp = ps_tile(0, "warm")
            for _ in range(6):
                nc.tensor.matmul(
                    wmp[:, :], wmx[:, :], wmw[:, :], start=True, stop=True
                )

            # ---- startup loads, greedy by first-use time ----
            # weight k-tile k is consumed at ~1.73us*k into phase 0; x strip j
            # (k-tiles 2j,2j+1) at ~3.46us*j. Emit in need order so the single
            # ~390GB/s DMA queue never gates the PE.
            wk = [None] * KT
            strips = [None] * len(STRIP_KS)

            head = wres.tile(
                [P, HEADW], dt.bfloat16, tag="head", name="head"
            )
            nc.sync.dma_start(out=head[:, :], in_=headp[:, :])
            wk[0] = head[:, 0:OS]
            head2 = wres.tile(
                [P, HEADW], dt.bfloat16, tag="head2", name="head2"
            )
            nc.sync.dma_start(out=head2[:, :], in_=head2p[:, :])
            wk[1] = head2[:, 0:OS]

            def load_w(k):
                w = wres.tile([P, OS], dt.bfloat16, tag=f"w{k}", name=f"w{k}")
                nc.sync.dma_start(out=w[:, :], in_=wT[k * P:(k + 1) * P, :])
                wk[k] = w

            def load_strip(j):
                k0, nk = STRIP_K0[j], STRIP_KS[j]
                st = xsl.tile(
                    [P, nk, TSLAB], dt.bfloat16, tag=f"x0s{j}", bufs=1,
                    name=f"x0s{j}",
                )
                nc.sync.dma_start(
                    out=st[:, :, :], in_=xview[:, k0:k0 + nk, 0:TSLAB]
                )
                strips[j] = st

            evs = [(1.73 * k, 0, k) for k in range(2, KT)]
            evs += [(1.73 * STRIP_K0[j], 1, j) for j in range(len(STRIP_KS))]
            evs.sort()
            for _, kind, idx in evs:
                if kind == 0:
                    load_w(idx)
                else:
                    load_strip(idx)

            biast = wres.tile([P, OS], dt.bfloat16, tag="bias", name="biast")
            nc.sync.dma_start(out=biast[:, :], in_=biasb[:, :])

            xs_tiles = {}

            def load_xs(s):
                xs = xsl.tile(
                    [P, KT, TSLAB], dt.bfloat16, tag="xs", bufs=2, name=f"xs{s}"
                )
                nc.sync.dma_start(
                    out=xs[:, :, :],
                    in_=xview[:, :, s * TSLAB:(s + 1) * TSLAB],
                )
                xs_tiles[s] = xs

            load_xs(1)

            import bisect

            def x0(k, tt):
                if k == 0:
                    return head[:, OS + tt * P:OS + (tt + 1) * P]
                if k == 1:
                    return head2[:, OS + tt * P:OS + (tt + 1) * P]
                j = bisect.bisect_right(STRIP_K0, k) - 1
                return strips[j][:, k - STRIP_K0[j], tt * P:(tt + 1) * P]

            # ---- phase 0: slab 0, k-outer over 8 concurrent PSUM groups ----
            G8 = [(0, 0), (0, 1), (0, 2), (1, 0), (1, 1), (1, 2), (2, 0), (2, 1)]
            ps8 = {g: ps_tile(g[1], f"p0_{g[0]}{g[1]}") for g in G8}
            for k in range(KT):
                for (tt, oc) in G8:
                    nc.tensor.matmul(
                        ps8[(tt, oc)][:, :],
                        x0(k, tt),
                        wk[k][:, oc * NQ:(oc + 1) * NQ],
                        start=(k == 0), stop=(k == KT - 1),
                    )
            for (tt, oc) in G8:
                drain(0, tt, ps8[(tt, oc)], oc)

            # ---- phase 0.5: slab-0 leftovers ----
            ps22 = ps_tile(2, "p05_22")
            for k in range(KT):
                nc.tensor.matmul(
                    ps22[:, :], x0(k, 2), wk[k][:, 2 * NQ:3 * NQ],
                    start=(k == 0), stop=(k == KT - 1),
                )
            drain(0, 2, ps22, 2)

            pss = [ps_tile(oc, f"p05_3{oc}") for oc in range(OCH)]
            for k in range(KT):
                for oc in range(OCH):
                    nc.tensor.matmul(
                        pss[oc][:, :], x0(k, 3),
                        wk[k][:, oc * NQ:(oc + 1) * NQ],
                        start=(k == 0), stop=(k == KT - 1),
                    )
            for oc in range(OCH):
                drain(0, 3, pss[oc], oc)

            # ---- slabs 1-7: steady oc-inner sweep ----
            for s in range(1, NSLAB):
                xs = xs_tiles.pop(s)
                if s + 1 < NSLAB:
                    load_xs(s + 1)
                for tt in range(TPS):
                    if s == NSLAB - 1 and tt == TPS - 1:
                        # Final group: per-oc sequential chains so earlier
                        # drains + y stores overlap later chains' matmuls;
                        # the very last chain runs as two half-column (N=256)
                        # chains so the final drain is half-size.
                        row = s * TSLAB + tt * P
                        for oc in range(OCH - 1):
                            ps = ps_tile(oc, f"ps_{s}_{tt}{oc}")
                            for k in range(KT):
                                nc.tensor.matmul(
                                    ps[:, :],
                                    xs[:, k, tt * P:(tt + 1) * P],
                                    wk[k][:, oc * NQ:(oc + 1) * NQ],
                                    start=(k == 0), stop=(k == KT - 1),
                                )
                            drain(s, tt, ps, oc)
                        oc = OCH - 1
                        ps = ps_tile(oc, f"ps_{s}_{tt}{oc}")
                        QW = NQ // 2
                        for h in range(2):
                            hlo = oc * NQ + h * QW
                            for k in range(KT):
                                nc.tensor.matmul(
                                    ps[:, h * QW:(h + 1) * QW],
                                    xs[:, k, tt * P:(tt + 1) * P],
                                    wk[k][:, hlo:hlo + QW],
                                    start=(k == 0), stop=(k == KT - 1),
                                )
                            ot = outp.tile(
                                [P, QW], dt.float32, tag="ot2", name="ot2"
                            )
                            nc.vector.tensor_add(
                                ot[:, :], ps[:, h * QW:(h + 1) * QW],
                                biast[:, hlo:hlo + QW],
                            )
                            nc.sync.dma_start(
                                out=y[row:row + P, hlo:hlo + QW],
                                in_=ot[:, :],
                            )
                        continue
                    pss = [ps_tile(oc, f"ps_{s}_{tt}{oc}") for oc in range(OCH)]
                    for k in range(KT):
                        for oc in range(OCH):
                            nc.tensor.matmul(
                                pss[oc][:, :],
                                xs[:, k, tt * P:(tt + 1) * P],
                                wk[k][:, oc * NQ:(oc + 1) * NQ],
                                start=(k == 0), stop=(k == KT - 1),
                            )
                    for oc in range(OCH):
                        drain(s, tt, pss[oc], oc)

    _strip_redundant_ldw(nc)
    nc.compile()
    return nc


def _prep_inputs(x, w_q, scales, bias):
    """Host-side shard + repack (dequant folded into the bf16 weight cast)."""
    xT = np.ascontiguousarray(x.T).astype(ml_dtypes.bfloat16)
    W = (w_q.astype(np.float32) * scales.astype(np.float32)).reshape(O, I)
    bias16 = bias.astype(ml_dtypes.bfloat16)
    in_maps = []
    for c in range(N_CORES):
        o0 = c * OS
        wT_c = np.ascontiguousarray(W[o0:o0 + OS].T.astype(ml_dtypes.bfloat16))
        biasb_c = np.ascontiguousarray(
            np.broadcast_to(bias16[o0:o0 + OS], (P, OS))
        )
        head_c = np.ascontiguousarray(
            np.concatenate([wT_c[0:P, :], xT[0:P, 0:TSLAB]], axis=1)
        )
        head2_c = np.ascontiguousarray(
            np.concatenate([wT_c[P:2 * P, :], xT[P:2 * P, 0:TSLAB]], axis=1)
        )
        in_maps.append(
            {"xT": xT, "wT": wT_c, "biasb": biasb_c,
             "head": head_c, "head2": head2_c}
        )
    return in_maps


def _get_nc():
    if "nc" not in _CACHE:
        _CACHE["nc"] = _build()
    return _CACHE["nc"]


def kernel(x, w_q, scales, bias):
    from concourse.bass_utils import run_bass_kernel_spmd

    nc = _get_nc()
    in_maps = _prep_inputs(
        np.asarray(x), np.asarray(w_q), np.asarray(scales), np.asarray(bias)
    )
    res = run_bass_kernel_spmd(nc, in_maps, list(range(N_CORES)))
    out = np.concatenate(
        [res.results[c]["y"] for c in range(N_CORES)], axis=1
    )
    return out.astype(np.float32)


# revision 29
# speedup vs baseline: 1.0012x; 1.0012x over previous
"""Trainium2 Bass kernel for DequantingLinear (GGML Q8_0 block-dequant + linear).

y = x @ (w_q * scales).reshape(O, I).T + bias

Sharding: tensor-parallel over out_features across 8 NeuronCores; x replicated.
The Q8_0 dequant (w = scale * int8_quant) is a per-element multiply folded into
host-side prep (bf16 round-to-nearest — numerically identical to doing the same
bf16 multiply on the DVE, but off the device's critical path). Each core runs a
bf16 GEMM over its output-column shard, accumulating in fp32 PSUM.

Startup: slab 0's x arrives as 12 k-strips (1KB DMA packets — the DMA engine is
packet-rate-bound at ~0.5 pkt/ns, so 256B packets would halve bandwidth), and
phase 0 runs k-outer across all 8 PSUM banks so the PE consumes weight k-tiles
at the DMA delivery rate. A handful of warm-up matmuls on memset tiles start
the HAM activity window early so real matmuls issue at the 2.4 GHz warm clock.
Slabs 1-7 run the steady oc-inner sweep at the PE issue roofline.

Host-side prep:
  - x   [T, I] f32   -> xT    [I, T] bf16  (replicated; contraction on partitions)
  - w_q*scales       -> wT    [I, O/8] bf16 per core (dequantized W^T shard)
  - bias [O] f32     -> biasb [128, O/8] bf16 per core (partition-broadcast)
"""

import numpy as np
import ml_dtypes

# Problem shape (hardcoded per contest rules).
T = 4096          # tokens (matmul M)
I = 3072          # in_features (contraction K)
O = 12288         # out_features (matmul N)
BLOCK = 32
N_CORES = 8
OS = O // N_CORES  # 1536 out features per core
P = 128           # partitions
KT = I // P       # 24 k-tiles
NQ = 512          # psum free-dim quantum (one bank)
OCH = OS // NQ    # 3 o-chunks per core
TSLAB = 512       # t columns per steady x slab
NSLAB = T // TSLAB   # 8 slabs
TPS = TSLAB // P     # 4 t-tiles per slab
# k=0's weight tile and x strip ship as one host-concatenated "head" tensor
# (single DMA issue + transfer on the critical first-matmul path). Remaining
# slab-0 x arrives as k-strips: one single-tile strip, then 2-tile strips.
HEADW = OS + TSLAB
STRIP_KS = [1] + [2] * 11
STRIP_K0 = [1 + sum(STRIP_KS[:j]) for j in range(len(STRIP_KS))]

_CACHE = {}


def _strip_redundant_ldw(nc):
    """Tile lowering prepends an InstLdweights to every InstMatmult. Walk each
    block in scheduled order tracking the weights AP currently loaded in the
    PE array; an InstLdweights identical to the resident one is redundant --
    remove it, migrating its sync waits/updates onto the next instruction."""
    removed = 0
    for f in nc.m.functions:
        for bb in f.blocks:
            insts = bb.instructions
            drop = []
            last_w = None
            for idx, ins in enumerate(insts):
                tn = type(ins).__name__
                if tn == "InstLdweights":
                    key = repr(ins.ins[0])
                    nxt = insts[idx + 1] if idx + 1 < len(insts) else None
                    if (
                        key == last_w
                        and nxt is not None
                        and type(nxt).__name__ == "InstMatmult"
                    ):
                        si = ins.sync_info
                        if si is not None and (si.on_wait or si.on_update):
                            nsi = nxt.sync_info
                            if nsi is None:
                                nxt.sync_info = si
                            else:
                                nsi.on_wait = list(si.on_wait) + list(nsi.on_wait)
                                nsi.on_update = (
                                    list(nsi.on_update) + list(si.on_update)
                                )
                        drop.append(idx)
                    else:
                        last_w = key
            for idx in reversed(drop):
                del insts[idx]
            removed += len(drop)
    return removed


def _build():
    import concourse.bacc as bacc
    import concourse.mybir as mybir
    from concourse.tile import TileContext

    nc = bacc.Bacc("TRN2", num_devices=N_CORES)
    dt = mybir.dt

    xT = nc.declare_dram_parameter("xT", [I, T], dt.bfloat16, isOutput=False)
    wT = nc.declare_dram_parameter("wT", [I, OS], dt.bfloat16, isOutput=False)
    headp = nc.declare_dram_parameter(
        "head", [P, HEADW], dt.bfloat16, isOutput=False
    )
    biasb = nc.declare_dram_parameter("biasb", [P, OS], dt.bfloat16, isOutput=False)
    y = nc.declare_dram_parameter("y", [T, OS], dt.float32, isOutput=True)

    PS_TAGS = ["ps0", "ps1", "ps2"]
    PS_BUFS = {"ps0": 3, "ps1": 3, "ps2": 2}   # 8 PSUM banks total

    with TileContext(nc) as tc:
        with (
            tc.tile_pool(name="wres", bufs=1) as wres,
            tc.tile_pool(name="xsl", bufs=2) as xsl,
            tc.tile_pool(name="outp", bufs=6) as outp,
            tc.tile_pool(name="psum", bufs=8, space="PSUM") as psum,
        ):
            xview = xT.rearrange("(k p) t -> p k t", p=P)

            def ps_tile(oc, name):
                tg = PS_TAGS[oc]
                return psum.tile(
                    [P, NQ], dt.float32, tag=tg, bufs=PS_BUFS[tg], name=name
                )

            def drain(s, tt, ps, oc):
                row = s * TSLAB + tt * P
                ot = outp.tile([P, NQ], dt.float32, tag="ot", name="ot")
                nc.vector.tensor_add(
                    ot[:, :], ps[:, :], biast[:, oc * NQ:(oc + 1) * NQ]
                )
                nc.sync.dma_start(
                    out=y[row:row + P, oc * NQ:(oc + 1) * NQ], in_=ot[:, :]
                )

            # ---- warm-up: start the PE HAM activity window early ----
            wmx = xsl.tile([P, P], dt.bfloat16, tag="wmx", bufs=1, name="wmx")
            nc.vector.memset(wmx[:, :], 0.0)
            wmw = xsl.tile([P, NQ], dt.bfloat16, tag="wmw", bufs=1, name="wmw")
            nc.vector.memset(wmw[:, :], 0.0)
            wmp = ps_tile(0, "warm")
            for _ in range(5):
                nc.tensor.matmul(
                    wmp[:, :], wmx[:, :], wmw[:, :], start=True, stop=True
                )

            # ---- startup loads, greedy by first-use time ----
            # weight k-tile k is consumed at ~1.73us*k into phase 0; x strip j
            # (k-tiles 2j,2j+1) at ~3.46us*j. Emit in need order so the single
            # ~390GB/s DMA queue never gates the PE.
            wk = [None] * KT
            strips = [None] * len(STRIP_KS)

            head = wres.tile(
                [P, HEADW], dt.bfloat16, tag="head", name="head"
            )
            nc.sync.dma_start(out=head[:, :], in_=headp[:, :])
            wk[0] = head[:, 0:OS]

            def load_w(k):
                w = wres.tile([P, OS], dt.bfloat16, tag=f"w{k}", name=f"w{k}")
                nc.sync.dma_start(out=w[:, :], in_=wT[k * P:(k + 1) * P, :])
                wk[k] = w

            def load_strip(j):
                k0, nk = STRIP_K0[j], STRIP_KS[j]
                st = xsl.tile(
                    [P, nk, TSLAB], dt.bfloat16, tag=f"x0s{j}", bufs=1,
                    name=f"x0s{j}",
                )
                nc.sync.dma_start(
                    out=st[:, :, :], in_=xview[:, k0:k0 + nk, 0:TSLAB]
                )
                strips[j] = st

            evs = [(1.73 * k, 0, k) for k in range(1, KT)]
            evs += [(1.73 * STRIP_K0[j], 1, j) for j in range(len(STRIP_KS))]
            evs.sort()
            for _, kind, idx in evs:
                if kind == 0:
                    load_w(idx)
                else:
                    load_strip(idx)

            biast = wres.tile([P, OS], dt.bfloat16, tag="bias", name="biast")
            nc.sync.dma_start(out=biast[:, :], in_=biasb[:, :])

            xs_tiles = {}

            def load_xs(s):
                xs = xsl.tile(
                    [P, KT, TSLAB], dt.bfloat16, tag="xs", bufs=2, name=f"xs{s}"
                )
                nc.sync.dma_start(
                    out=xs[:, :, :],
                    in_=xview[:, :, s * TSLAB:(s + 1) * TSLAB],
                )
                xs_tiles[s] = xs

            load_xs(1)

            import bisect

            def x0(k, tt):
                if k == 0:
                    return head[:, OS + tt * P:OS + (tt + 1) * P]
                j = bisect.bisect_right(STRIP_K0, k) - 1
                return strips[j][:, k - STRIP_K0[j], tt * P:(tt + 1) * P]

            # ---- phase 0: slab 0, k-outer over 8 concurrent PSUM groups ----
            G8 = [(0, 0), (0, 1), (0, 2), (1, 0), (1, 1), (1, 2), (2, 0), (2, 1)]
            ps8 = {g: ps_tile(g[1], f"p0_{g[0]}{g[1]}") for g in G8}
            for k in range(KT):
                for (tt, oc) in G8:
                    nc.tensor.matmul(
                        ps8[(tt, oc)][:, :],
                        x0(k, tt),
                        wk[k][:, oc * NQ:(oc + 1) * NQ],
                        start=(k == 0), stop=(k == KT - 1),
                    )
            for (tt, oc) in G8:
                drain(0, tt, ps8[(tt, oc)], oc)

            # ---- phase 0.5: slab-0 leftovers ----
            ps22 = ps_tile(2, "p05_22")
            for k in range(KT):
                nc.tensor.matmul(
                    ps22[:, :], x0(k, 2), wk[k][:, 2 * NQ:3 * NQ],
                    start=(k == 0), stop=(k == KT - 1),
                )
            drain(0, 2, ps22, 2)

            pss = [ps_tile(oc, f"p05_3{oc}") for oc in range(OCH)]
            for k in range(KT):
                for oc in range(OCH):
                    nc.tensor.matmul(
                        pss[oc][:, :], x0(k, 3),
                        wk[k][:, oc * NQ:(oc + 1) * NQ],
                        start=(k == 0), stop=(k == KT - 1),
                    )
            for oc in range(OCH):
                drain(0, 3, pss[oc], oc)

            # ---- slabs 1-7: steady oc-inner sweep ----
            for s in range(1, NSLAB):
                xs = xs_tiles.pop(s)
                if s + 1 < NSLAB:
                    load_xs(s + 1)
                for tt in range(TPS):
                    if s == NSLAB - 1 and tt == TPS - 1:
                        # Final group: per-oc sequential chains so earlier
                        # drains + y stores overlap later chains' matmuls;
                        # the very last chain runs as two half-column (N=256)
                        # chains so the final drain is half-size.
                        row = s * TSLAB + tt * P
                        for oc in range(OCH - 1):
                            ps = ps_tile(oc, f"ps_{s}_{tt}{oc}")
                            for k in range(KT):
                                nc.tensor.matmul(
                                    ps[:, :],
                                    xs[:, k, tt * P:(tt + 1) * P],
                                    wk[k][:, oc * NQ:(oc + 1) * NQ],
                                    start=(k == 0), stop=(k == KT - 1),
                                )
                            drain(s, tt, ps, oc)
                        oc = OCH - 1
                        ps = ps_tile(oc, f"ps_{s}_{tt}{oc}")
                        QW = NQ // 2
                        for h in range(2):
                            hlo = oc * NQ + h * QW
                            for k in range(KT):
                                nc.tensor.matmul(
                                    ps[:, h * QW:(h + 1) * QW],
                                    xs[:, k, tt * P:(tt + 1) * P],
                                    wk[k][:, hlo:hlo + QW],
                                    start=(k == 0), stop=(k == KT - 1),
                                )
                            ot = outp.tile(
                                [P, QW], dt.float32, tag="ot2", name="ot2"
                            )
                            nc.vector.tensor_add(
                                ot[:, :], ps[:, h * QW:(h + 1) * QW],
                                biast[:, hlo:hlo + QW],
                            )
                            nc.sync.dma_start(
                                out=y[row:row + P, hlo:hlo + QW],
                                in_=ot[:, :],
                            )
                        continue
                    pss = [ps_tile(oc, f"ps_{s}_{tt}{oc}") for oc in range(OCH)]
                    for k in range(KT):
                        for oc in range(OCH):
                            nc.tensor.matmul(
                                pss[oc][:, :],
                                xs[:, k, tt * P:(tt + 1) * P],
                                wk[k][:, oc * NQ:(oc + 1) * NQ],
                                start=(k == 0), stop=(k == KT - 1),
                            )
                    for oc in range(OCH):
                        drain(s, tt, pss[oc], oc)

    _strip_redundant_ldw(nc)
    nc.compile()
    return nc


def _prep_inputs(x, w_q, scales, bias):
    """Host-side shard + repack (dequant folded into the bf16 weight cast)."""
    xT = np.ascontiguousarray(x.T).astype(ml_dtypes.bfloat16)
    W = (w_q.astype(np.float32) * scales.astype(np.float32)).reshape(O, I)
    bias16 = bias.astype(ml_dtypes.bfloat16)
    in_maps = []
    for c in range(N_CORES):
        o0 = c * OS
        wT_c = np.ascontiguousarray(W[o0:o0 + OS].T.astype(ml_dtypes.bfloat16))
        biasb_c = np.ascontiguousarray(
            np.broadcast_to(bias16[o0:o0 + OS], (P, OS))
        )
        head_c = np.ascontiguousarray(
            np.concatenate([wT_c[0:P, :], xT[0:P, 0:TSLAB]], axis=1)
        )
        in_maps.append(
            {"xT": xT, "wT": wT_c, "biasb": biasb_c, "head": head_c}
        )
    return in_maps


def _get_nc():
    if "nc" not in _CACHE:
        _CACHE["nc"] = _build()
    return _CACHE["nc"]


def kernel(x, w_q, scales, bias):
    from concourse.bass_utils import run_bass_kernel_spmd

    nc = _get_nc()
    in_maps = _prep_inputs(
        np.asarray(x), np.asarray(w_q), np.asarray(scales), np.asarray(bias)
    )
    res = run_bass_kernel_spmd(nc, in_maps, list(range(N_CORES)))
    out = np.concatenate(
        [res.results[c]["y"] for c in range(N_CORES)], axis=1
    )
    return out.astype(np.float32)
